# revision 1
# baseline (speedup 1.0000x reference)
"""Causal self-attention (B=4, N=2048, D=1024, H=16) on 8 TRN2 NeuronCores.

Sharding: head-parallel — core i computes heads {2i, 2i+1} for all batches
(QKV projection + attention), then one 8-rank AllToAll reshards from
head-split to token-split, and each core runs the output projection for its
1024-token slice. No partial-sum collective is needed: the AllToAll gives
each core the full concat-head activation for its tokens.

All matmuls run in float32r (TF32-like full-rate fp32, ~1.5e-4 rounding).
Attention uses the score-transposed (ST) layout [k, q] so no P transposes
are needed; softmax denominators come from a ones-column appended to V
(PV matmul M=65), and scores are small enough (~N(0,1)) that max-subtraction
is unnecessary.
"""

import sys

for _p in ("/opt/trn_rl_repo", "/root/.axon_site/_ro/trn_rl_repo"):
    if _p not in sys.path:
        sys.path.append(_p)

import numpy as np

import concourse.bass as bass
import concourse.tile as tile
from concourse import bacc, mybir
from concourse.bass_utils import run_bass_kernel_spmd
from concourse.masks import make_identity

dt = mybir.dt

B, N, D, H, HD = 4, 2048, 1024, 16, 64
BN = B * N                      # 8192 flattened tokens
NCORES = 8
HL = H // NCORES                # 2 local heads per core
F = HL * HD                     # 128 local feats
SCALE = HD ** -0.5              # 0.125

KT = D // 128                   # 8 contraction tiles for the projections
TCH = BN // 512                 # 16 token chunks of 512
TPB = N // 512                  # 4 token chunks per batch
KPB = N // 128                  # 16 k-tiles per batch
TT = BN // 128                  # 64 token tiles of 128

_compiled = None


def _build():
    nc = bacc.Bacc("TRN2", target_bir_lowering=False, debug=False,
                   num_devices=NCORES)

    xT = nc.declare_dram_parameter("xT", [D, BN], dt.float32, isOutput=False)
    wqkv_t = nc.declare_dram_parameter("wqkv_t", [D, 3 * F], dt.float32, isOutput=False)
    bqk = nc.declare_dram_parameter("bqk", [F, 2], dt.float32, isOutput=False)
    bv = nc.declare_dram_parameter("bv", [F, 1], dt.float32, isOutput=False)
    wout_t = nc.declare_dram_parameter("wout_t", [D, D], dt.float32, isOutput=False)
    bout_rep = nc.declare_dram_parameter("bout_rep", [128, D], dt.float32, isOutput=False)
    masks = nc.declare_dram_parameter("masks", [4, 128, 512], dt.float32, isOutput=False)
    ones_col = nc.declare_dram_parameter("ones_col", [128, HL], dt.float32, isOutput=False)
    out = nc.declare_dram_parameter("out", [BN // NCORES, D], dt.float32, isOutput=True)

    f32, f32r = dt.float32, dt.float32r

    with tile.TileContext(nc) as tc:
        with (
            tc.tile_pool(name="const", bufs=1) as const,
            tc.tile_pool(name="attn", bufs=1) as attn_pool,
            tc.tile_pool(name="dram", bufs=1, space="DRAM") as dram,
        ):
            # --- constants ---
            wqkv_sb = const.tile([128, KT, 3 * F], f32r)
            for kt in range(KT):
                nc.sync.dma_start(
                    out=wqkv_sb[:, kt, :],
                    in_=wqkv_t[128 * kt:128 * (kt + 1), :].bitcast(f32r))
            bqk_sb = const.tile([F, 2], f32)
            nc.sync.dma_start(out=bqk_sb, in_=bqk[:])
            bv_sb = const.tile([F, 1], f32)
            nc.sync.dma_start(out=bv_sb, in_=bv[:])
            masks_sb = const.tile([128, 4, 512], f32)
            for j in range(4):
                nc.sync.dma_start(out=masks_sb[:, j, :], in_=masks[j])
            ident = const.tile([128, 128], f32)
            make_identity(nc, ident)

            attnT_sb = attn_pool.tile([128, BN], f32)      # normalized O^T, all heads

            a2a_in = dram.tile([NCORES, F, BN // NCORES], f32)
            a2a_out = dram.tile([NCORES, F, BN // NCORES], f32)

            with (
                tc.tile_pool(name="qkvT", bufs=1) as qkvT,
                tc.tile_pool(name="xt", bufs=2) as xt_pool,
                tc.tile_pool(name="vt", bufs=2) as vt_pool,
                tc.tile_pool(name="pt", bufs=3) as pt_pool,
                tc.tile_pool(name="nrm", bufs=2) as nrm,
                tc.tile_pool(name="ps_qkv", bufs=2, space="PSUM") as ps_qkv,
                tc.tile_pool(name="ps_tr", bufs=1, space="PSUM") as ps_tr,
                tc.tile_pool(name="ps_s", bufs=3, space="PSUM") as ps_s,
                tc.tile_pool(name="ps_o", bufs=2, space="PSUM") as ps_o,
            ):
                qT_sb = qkvT.tile([F, BN], f32r)
                kT_sb = qkvT.tile([F, BN], f32r)
                v1_sb = qkvT.tile([128, TT, HL * (HD + 1)], f32r)

                for b in range(B):
                    # ---- phase 1: qkv projection for batch b ----
                    for tc_i in range(TPB):
                        tch = TPB * b + tc_i
                        sl = slice(512 * tch, 512 * (tch + 1))
                        xt = xt_pool.tile([128, KT, 512], f32r, tag="xt")
                        for kt in range(KT):
                            nc.sync.dma_start(
                                out=xt[:, kt, :],
                                in_=xT[128 * kt:128 * (kt + 1), sl].bitcast(f32r))
                        # q then k then v — one accumulating psum each
                        for which, dst in ((0, qT_sb), (1, kT_sb)):
                            ps = ps_qkv.tile([128, 512], f32, tag="qkv")
                            for kt in range(KT):
                                nc.tensor.matmul(
                                    ps,
                                    wqkv_sb[:, kt, F * which:F * (which + 1)],
                                    xt[:, kt, :],
                                    start=(kt == 0), stop=(kt == KT - 1))
                            nc.vector.tensor_scalar_add(
                                dst[:, sl], ps, bqk_sb[:, which:which + 1])
                        ps = ps_qkv.tile([128, 512], f32, tag="qkv")
                        for kt in range(KT):
                            nc.tensor.matmul(
                                ps, wqkv_sb[:, kt, 2 * F:3 * F], xt[:, kt, :],
                                start=(kt == 0), stop=(kt == KT - 1))
                        vt = vt_pool.tile([128, 512], f32, tag="vt")
                        nc.vector.tensor_scalar_add(vt, ps, bv_sb)
                        # transpose to natural V, interleave ones columns
                        for j in range(4):
                            tt = 4 * tch + j
                            ptr = ps_tr.tile([128, 128], f32, tag="tr")
                            nc.tensor.transpose(ptr, vt[:, 128 * j:128 * (j + 1)], ident)
                            nc.scalar.activation(
                                out=v1_sb[:, tt, :].rearrange(
                                    "p (h e) -> p h e", h=HL)[:, :, 0:HD],
                                in_=ptr.rearrange("p (h d) -> p h d", h=HL),
                                func=mybir.ActivationFunctionType.Copy)
                            nc.sync.dma_start(
                                out=v1_sb[:, tt, :].rearrange(
                                    "p (h e) -> p h e", h=HL)[:, :, HD:HD + 1],
                                in_=ones_col[:].bitcast(f32r).unsqueeze(2))

                    # ---- phase 2: attention for batch b, both heads ----
                    for h in range(HL):
                        hsl = slice(HD * h, HD * (h + 1))
                        for qc in range(TPB):
                            qsl = slice(N * b + 512 * qc, N * b + 512 * (qc + 1))
                            po = ps_o.tile([HD + 1, 512], f32, tag="o")
                            nkt = 4 * qc + 4
                            for kt in range(nkt):
                                ks = ps_s.tile([128, 512], f32, tag="s")
                                nc.tensor.matmul(
                                    ks,
                                    kT_sb[hsl, N * b + 128 * kt:N * b + 128 * (kt + 1)],
                                    qT_sb[hsl, qsl],
                                    start=True, stop=True)
                                pt = pt_pool.tile([128, 512], f32r, tag="pt")
                                nc.scalar.activation(
                                    out=pt, in_=ks,
                                    func=mybir.ActivationFunctionType.Exp,
                                    scale=SCALE)
                                if kt >= 4 * qc:
                                    nc.vector.tensor_mul(
                                        pt, pt, masks_sb[:, kt - 4 * qc, :])
                                nc.tensor.matmul(
                                    po,
                                    v1_sb[:, KPB * b + kt,
                                          (HD + 1) * h:(HD + 1) * (h + 1)],
                                    pt,
                                    start=(kt == 0), stop=(kt == nkt - 1))
                            recip = nrm.tile([1, 512], f32, tag="recip")
                            nc.vector.reciprocal(recip, po[HD:HD + 1, :])
                            bc = nrm.tile([HD, 512], f32, tag="bc")
                            nc.gpsimd.partition_broadcast(bc, recip)
                            nc.vector.tensor_mul(
                                attnT_sb[HD * h:HD * (h + 1), qsl].bitcast(f32),
                                po[0:HD, :], bc)

            # ---- phase 3: AllToAll reshard (head-split -> token-split) ----
            TOK = BN // NCORES
            for j in range(NCORES):
                nc.sync.dma_start(out=a2a_in[j],
                                  in_=attnT_sb[:, TOK * j:TOK * (j + 1)])
            nc.gpsimd.collective_compute(
                "AllToAll",
                mybir.AluOpType.bypass,
                replica_groups=[list(range(NCORES))],
                ins=[a2a_in.opt()],
                outs=[a2a_out.opt()],
            )

            # ---- phase 4: output projection for my 1024 tokens ----
            with (
                tc.tile_pool(name="oproj", bufs=1) as oproj,
                tc.tile_pool(name="osb", bufs=2) as osb,
                tc.tile_pool(name="ps_out", bufs=2, space="PSUM") as ps_out,
            ):
                wout_sb = oproj.tile([128, KT, D], f32r)
                for kt in range(KT):
                    nc.sync.dma_start(
                        out=wout_sb[:, kt, :],
                        in_=wout_t[128 * kt:128 * (kt + 1), :].bitcast(f32r))
                bout_sb = oproj.tile([128, D], f32)
                nc.sync.dma_start(out=bout_sb, in_=bout_rep[:])
                ot_sb = oproj.tile([128, KT, TOK], f32r)
                for kt in range(KT):
                    nc.sync.dma_start(out=ot_sb[:, kt, :],
                                      in_=a2a_out[kt].bitcast(f32r))

                for mt in range(TOK // 128):
                    o_sb = osb.tile([128, D], f32, tag="osb")
                    for nb in range(2):
                        ps = ps_out.tile([128, 512], f32, tag="out")
                        for kt in range(KT):
                            nc.tensor.matmul(
                                ps,
                                ot_sb[:, kt, 128 * mt:128 * (mt + 1)],
                                wout_sb[:, kt, 512 * nb:512 * (nb + 1)],
                                start=(kt == 0), stop=(kt == KT - 1))
                        nc.vector.tensor_add(
                            o_sb[:, 512 * nb:512 * (nb + 1)], ps,
                            bout_sb[:, 512 * nb:512 * (nb + 1)])
                    nc.sync.dma_start(out=out[128 * mt:128 * (mt + 1), :], in_=o_sb)

    nc.compile()
    return nc


def _prep_inputs(x, w_qkv, b_qkv, w_out, b_out):
    x = np.asarray(x, dtype=np.float32)
    w_qkv = np.asarray(w_qkv, dtype=np.float32)
    b_qkv = np.asarray(b_qkv, dtype=np.float32)
    w_out = np.asarray(w_out, dtype=np.float32)
    b_out = np.asarray(b_out, dtype=np.float32)

    xT = np.ascontiguousarray(x.reshape(BN, D).T)
    wout_t = np.ascontiguousarray(w_out.T)
    bout_rep = np.ascontiguousarray(np.broadcast_to(b_out[None, :], (128, D)))
    ones_col = np.ones((128, HL), dtype=np.float32)

    mk = np.zeros((4, 128, 512), dtype=np.float32)
    for j in range(4):
        kk = 128 * j + np.arange(128)[:, None]
        qq = np.arange(512)[None, :]
        mk[j] = (kk <= qq).astype(np.float32)

    in_maps = []
    for i in range(NCORES):
        fs = slice(F * i, F * (i + 1))
        wq, wk, wv = w_qkv[0:D][fs], w_qkv[D:2 * D][fs], w_qkv[2 * D:3 * D][fs]
        wqkv_t = np.ascontiguousarray(np.concatenate([wq, wk, wv], axis=0).T)
        bqk = np.ascontiguousarray(
            np.stack([b_qkv[0:D][fs], b_qkv[D:2 * D][fs]], axis=1))
        bv = np.ascontiguousarray(b_qkv[2 * D:3 * D][fs][:, None])
        in_maps.append({
            "xT": xT, "wqkv_t": wqkv_t, "bqk": bqk, "bv": bv,
            "wout_t": wout_t, "bout_rep": bout_rep, "masks": mk,
            "ones_col": ones_col,
        })
    return in_maps


def kernel(x, w_qkv, b_qkv, w_out, b_out, _results_hook=None):
    global _compiled
    if _compiled is None:
        _compiled = _build()
    in_maps = _prep_inputs(x, w_qkv, b_qkv, w_out, b_out)
    res = run_bass_kernel_spmd(_compiled, in_maps, core_ids=list(range(NCORES)))
    if _results_hook is not None:
        _results_hook(res)
    full = np.concatenate(
        [res.results[i]["out"] for i in range(NCORES)], axis=0)
    return full.reshape(B, N, D)


# revision 2
# speedup vs baseline: 1.3189x; 1.3189x over previous
"""Causal self-attention (B=4, N=2048, D=1024, H=16) on 8 TRN2 NeuronCores.

Sharding: head-parallel — core i computes heads {2i, 2i+1} for all batches
(QKV projection + attention), then 8-rank AllToAll collectives (one per
batch, overlapped with the next batch's attention) reshard from head-split
to token-split, and each core runs the output projection for its 1024
tokens. The AllToAll gives each core the full concat-head activation for
its tokens, so no partial-sum collective is needed.

All matmuls run in float32r (TF32-like full-rate fp32, ~1.5e-4 rounding).
Attention uses the score-transposed (ST) layout [k, q] so no P transposes
are needed; softmax denominators come from a ones-column appended to V
(PV matmul M=65), and scores are small enough (~N(0,1)) that max-subtraction
is unnecessary. Projection matmuls for batch b+1 are emitted interleaved
with attention groups of batch b to keep the PE queue dense (HAM warmth).
"""

import sys

for _p in ("/opt/trn_rl_repo", "/root/.axon_site/_ro/trn_rl_repo"):
    if _p not in sys.path:
        sys.path.append(_p)

import numpy as np

import concourse.bass as bass
import concourse.tile as tile
from concourse import bacc, mybir
from concourse.bass_utils import run_bass_kernel_spmd
from concourse.masks import make_identity

dt = mybir.dt

B, N, D, H, HD = 4, 2048, 1024, 16, 64
BN = B * N                      # 8192 flattened tokens
NCORES = 8
HL = H // NCORES                # 2 local heads per core
F = HL * HD                     # 128 local feats
SCALE = HD ** -0.5              # 0.125

KT = D // 128                   # 8 contraction tiles for the projections
TPB = N // 512                  # 4 token chunks per batch
KPB = N // 128                  # 16 k-tiles per batch
TT = BN // 128                  # 64 token tiles of 128
TOK = BN // NCORES              # 1024 tokens per core post-reshard
CH = N // NCORES                # 256 tokens per core per batch chunk

_compiled = None


def _build():
    nc = bacc.Bacc("TRN2", target_bir_lowering=False, debug=False,
                   num_devices=NCORES)

    xT = nc.declare_dram_parameter("xT", [D, BN], dt.float32, isOutput=False)
    wqkv_t = nc.declare_dram_parameter("wqkv_t", [D, 3 * F], dt.float32, isOutput=False)
    bqk = nc.declare_dram_parameter("bqk", [F, 2], dt.float32, isOutput=False)
    bv = nc.declare_dram_parameter("bv", [F, 1], dt.float32, isOutput=False)
    wout_t = nc.declare_dram_parameter("wout_t", [D, D], dt.float32, isOutput=False)
    bout_rep = nc.declare_dram_parameter("bout_rep", [128, D], dt.float32, isOutput=False)
    masks = nc.declare_dram_parameter("masks", [4, 128, 512], dt.float32, isOutput=False)
    ones_col = nc.declare_dram_parameter("ones_col", [128, HL], dt.float32, isOutput=False)
    out = nc.declare_dram_parameter("out", [TOK, D], dt.float32, isOutput=True)

    f32, f32r = dt.float32, dt.float32r

    with tile.TileContext(nc) as tc:
        with (
            tc.tile_pool(name="const", bufs=1) as const,
            tc.tile_pool(name="attn", bufs=1) as attn_pool,
            tc.tile_pool(name="dram", bufs=1, space="DRAM") as dram,
        ):
            # --- constants ---
            wqkv_sb = const.tile([128, KT, 3 * F], f32r)
            for kt in range(KT):
                nc.sync.dma_start(
                    out=wqkv_sb[:, kt, :],
                    in_=wqkv_t[128 * kt:128 * (kt + 1), :].bitcast(f32r))
            bqk_sb = const.tile([F, 2], f32)
            nc.sync.dma_start(out=bqk_sb, in_=bqk[:])
            bv_sb = const.tile([F, 1], f32)
            nc.sync.dma_start(out=bv_sb, in_=bv[:])
            masks_sb = const.tile([128, 4, 512], f32)
            for j in range(4):
                nc.sync.dma_start(out=masks_sb[:, j, :], in_=masks[j])
            ident = const.tile([128, 128], f32)
            make_identity(nc, ident)

            attnT_sb = attn_pool.tile([128, BN], f32)   # normalized O^T

            a2a_in = [dram.tile([NCORES, F, CH], f32, name=f"a2a_in{b}")
                      for b in range(B)]
            a2a_out = [dram.tile([NCORES, F, CH], f32, name=f"a2a_out{b}")
                       for b in range(B)]

            with (
                tc.tile_pool(name="qkvT", bufs=1) as qkvT,
                tc.tile_pool(name="xt", bufs=2) as xt_pool,
                tc.tile_pool(name="vt", bufs=2) as vt_pool,
                tc.tile_pool(name="pt", bufs=3) as pt_pool,
                tc.tile_pool(name="nrm", bufs=2) as nrm,
                tc.tile_pool(name="ps_qkv", bufs=2, space="PSUM") as ps_qkv,
                tc.tile_pool(name="ps_tr", bufs=1, space="PSUM") as ps_tr,
                tc.tile_pool(name="ps_s", bufs=3, space="PSUM") as ps_s,
                tc.tile_pool(name="ps_o", bufs=2, space="PSUM") as ps_o,
            ):
                qT_sb = qkvT.tile([F, BN], f32r)
                kT_sb = qkvT.tile([F, BN], f32r)
                v1_sb = qkvT.tile([128, TT, HL * (HD + 1)], f32r)

                def proj_chunk(tch):
                    """QKV projection for one 512-token chunk."""
                    sl = slice(512 * tch, 512 * (tch + 1))
                    xt = xt_pool.tile([128, KT, 512], f32r, tag="xt")
                    for kt in range(KT):
                        nc.sync.dma_start(
                            out=xt[:, kt, :],
                            in_=xT[128 * kt:128 * (kt + 1), sl].bitcast(f32r))
                    for which, dst in ((0, qT_sb), (1, kT_sb)):
                        ps = ps_qkv.tile([128, 512], f32, tag="qkv")
                        for kt in range(KT):
                            nc.tensor.matmul(
                                ps,
                                wqkv_sb[:, kt, F * which:F * (which + 1)],
                                xt[:, kt, :],
                                start=(kt == 0), stop=(kt == KT - 1))
                        nc.vector.tensor_scalar_add(
                            dst[:, sl], ps, bqk_sb[:, which:which + 1])
                    ps = ps_qkv.tile([128, 512], f32, tag="qkv")
                    for kt in range(KT):
                        nc.tensor.matmul(
                            ps, wqkv_sb[:, kt, 2 * F:3 * F], xt[:, kt, :],
                            start=(kt == 0), stop=(kt == KT - 1))
                    vt = vt_pool.tile([128, 512], f32, tag="vt")
                    nc.vector.tensor_scalar_add(vt, ps, bv_sb)
                    for j in range(4):
                        tt = 4 * tch + j
                        ptr = ps_tr.tile([128, 128], f32, tag="tr")
                        nc.tensor.transpose(ptr, vt[:, 128 * j:128 * (j + 1)], ident)
                        nc.scalar.activation(
                            out=v1_sb[:, tt, :].rearrange(
                                "p (h e) -> p h e", h=HL)[:, :, 0:HD],
                            in_=ptr.rearrange("p (h d) -> p h d", h=HL),
                            func=mybir.ActivationFunctionType.Copy)
                        nc.sync.dma_start(
                            out=v1_sb[:, tt, :].rearrange(
                                "p (h e) -> p h e", h=HL)[:, :, HD:HD + 1],
                            in_=ones_col[:].bitcast(f32r).unsqueeze(2))

                def attn_group(b, h, qc):
                    """Scores+softmax+PV for one (head, 512-query chunk)."""
                    hsl = slice(HD * h, HD * (h + 1))
                    qsl = slice(N * b + 512 * qc, N * b + 512 * (qc + 1))
                    po = ps_o.tile([HD + 1, 512], f32, tag="o")
                    nkt = 4 * qc + 4
                    for kt in range(nkt):
                        ks = ps_s.tile([128, 512], f32, tag="s")
                        nc.tensor.matmul(
                            ks,
                            kT_sb[hsl, N * b + 128 * kt:N * b + 128 * (kt + 1)],
                            qT_sb[hsl, qsl],
                            start=True, stop=True)
                        pt = pt_pool.tile([128, 512], f32r, tag="pt")
                        nc.scalar.activation(
                            out=pt, in_=ks,
                            func=mybir.ActivationFunctionType.Exp,
                            scale=SCALE)
                        if kt >= 4 * qc:
                            nc.vector.tensor_mul(
                                pt, pt, masks_sb[:, kt - 4 * qc, :])
                        nc.tensor.matmul(
                            po,
                            v1_sb[:, KPB * b + kt,
                                  (HD + 1) * h:(HD + 1) * (h + 1)],
                            pt,
                            start=(kt == 0), stop=(kt == nkt - 1))
                    recip = nrm.tile([1, 512], f32, tag="recip")
                    nc.vector.reciprocal(recip, po[HD:HD + 1, :])
                    bc = nrm.tile([HD, 512], f32, tag="bc")
                    nc.gpsimd.partition_broadcast(bc, recip)
                    nc.vector.tensor_mul(
                        attnT_sb[HD * h:HD * (h + 1), qsl].bitcast(f32),
                        po[0:HD, :], bc)

                def a2a_chunk(b):
                    """Ship batch b's attnT through the AllToAll."""
                    for j in range(NCORES):
                        c0 = N * b + CH * j
                        nc.sync.dma_start(out=a2a_in[b][j],
                                          in_=attnT_sb[:, c0:c0 + CH])
                    nc.gpsimd.collective_compute(
                        "AllToAll",
                        mybir.AluOpType.bypass,
                        replica_groups=[list(range(NCORES))],
                        ins=[a2a_in[b].opt()],
                        outs=[a2a_out[b].opt()],
                    )

                # ---- emission: proj(0), then attn(b) interleaved with
                # proj(b+1); a2a(b) fires right after attn(b) ----
                for tch in range(TPB):
                    proj_chunk(tch)
                for b in range(B):
                    groups = [(h, qc) for qc in range(TPB) for h in range(HL)]
                    for gi, (h, qc) in enumerate(groups):
                        attn_group(b, h, qc)
                        if b + 1 < B and gi % 2 == 0:
                            proj_chunk(TPB * (b + 1) + gi // 2)
                    a2a_chunk(b)

            # ---- output projection for my 1024 tokens (4 x 256 chunks) ----
            with (
                tc.tile_pool(name="oproj", bufs=1) as oproj,
                tc.tile_pool(name="osb", bufs=2) as osb,
                tc.tile_pool(name="ps_out", bufs=2, space="PSUM") as ps_out,
            ):
                wout_sb = oproj.tile([128, KT, D], f32r)
                for kt in range(KT):
                    nc.sync.dma_start(
                        out=wout_sb[:, kt, :],
                        in_=wout_t[128 * kt:128 * (kt + 1), :].bitcast(f32r))
                bout_sb = oproj.tile([128, D], f32)
                nc.sync.dma_start(out=bout_sb, in_=bout_rep[:])
                ot_sb = oproj.tile([128, KT, TOK], f32r)
                for b in range(B):
                    for kt in range(KT):
                        nc.sync.dma_start(
                            out=ot_sb[:, kt, CH * b:CH * (b + 1)],
                            in_=a2a_out[b][kt].bitcast(f32r))

                for mt in range(TOK // 128):
                    o_sb = osb.tile([128, D], f32, tag="osb")
                    for nb in range(2):
                        ps = ps_out.tile([128, 512], f32, tag="out")
                        for kt in range(KT):
                            nc.tensor.matmul(
                                ps,
                                ot_sb[:, kt, 128 * mt:128 * (mt + 1)],
                                wout_sb[:, kt, 512 * nb:512 * (nb + 1)],
                                start=(kt == 0), stop=(kt == KT - 1))
                        nc.vector.tensor_add(
                            o_sb[:, 512 * nb:512 * (nb + 1)], ps,
                            bout_sb[:, 512 * nb:512 * (nb + 1)])
                    nc.sync.dma_start(out=out[128 * mt:128 * (mt + 1), :], in_=o_sb)

    nc.compile()
    return nc


def _prep_inputs(x, w_qkv, b_qkv, w_out, b_out):
    x = np.asarray(x, dtype=np.float32)
    w_qkv = np.asarray(w_qkv, dtype=np.float32)
    b_qkv = np.asarray(b_qkv, dtype=np.float32)
    w_out = np.asarray(w_out, dtype=np.float32)
    b_out = np.asarray(b_out, dtype=np.float32)

    xT = np.ascontiguousarray(x.reshape(BN, D).T)
    wout_t = np.ascontiguousarray(w_out.T)
    bout_rep = np.ascontiguousarray(np.broadcast_to(b_out[None, :], (128, D)))
    ones_col = np.ones((128, HL), dtype=np.float32)

    mk = np.zeros((4, 128, 512), dtype=np.float32)
    for j in range(4):
        kk = 128 * j + np.arange(128)[:, None]
        qq = np.arange(512)[None, :]
        mk[j] = (kk <= qq).astype(np.float32)

    in_maps = []
    for i in range(NCORES):
        fs = slice(F * i, F * (i + 1))
        wq, wk, wv = w_qkv[0:D][fs], w_qkv[D:2 * D][fs], w_qkv[2 * D:3 * D][fs]
        wqkv_t = np.ascontiguousarray(np.concatenate([wq, wk, wv], axis=0).T)
        bqk_np = np.ascontiguousarray(
            np.stack([b_qkv[0:D][fs], b_qkv[D:2 * D][fs]], axis=1))
        bv_np = np.ascontiguousarray(b_qkv[2 * D:3 * D][fs][:, None])
        in_maps.append({
            "xT": xT, "wqkv_t": wqkv_t, "bqk": bqk_np, "bv": bv_np,
            "wout_t": wout_t, "bout_rep": bout_rep, "masks": mk,
            "ones_col": ones_col,
        })
    return in_maps


def kernel(x, w_qkv, b_qkv, w_out, b_out, _results_hook=None):
    global _compiled
    if _compiled is None:
        _compiled = _build()
    in_maps = _prep_inputs(x, w_qkv, b_qkv, w_out, b_out)
    res = run_bass_kernel_spmd(_compiled, in_maps, core_ids=list(range(NCORES)))
    if _results_hook is not None:
        _results_hook(res)
    full = np.empty((B, N, D), dtype=np.float32)
    for i in range(NCORES):
        o = res.results[i]["out"]            # [1024, D]: 4 chunks of 256
        for b in range(B):
            full[b, CH * i:CH * (i + 1), :] = o[CH * b:CH * (b + 1)]
    return full


# revision 4
# speedup vs baseline: 1.5345x; 1.1635x over previous
"""Causal self-attention (B=4, N=2048, D=1024, H=16) on 8 TRN2 NeuronCores.

Sharding: head-parallel — core i computes heads {2i, 2i+1} for all batches
(QKV projection + attention), then 8-rank AllToAll collectives (one per
batch, overlapped with the next batch's attention) reshard from head-split
to token-split, and each core runs the output projection for its 1024
tokens. The AllToAll gives each core the full concat-head activation for
its tokens, so no partial-sum collective is needed.

Matmuls run in bf16 with fp32 PSUM accumulation (measured ~3e-3 max rel
error end-to-end; bf16 streams at 1 cycle/row on the PE vs ~1.8 for
fp32r). Attention uses the score-transposed (ST) layout [k, q] so no P
transposes are needed; softmax denominators come from a ones-column
appended to V (PV matmul M=65), and scores are ~N(0,1) so max-subtraction
is unnecessary. Projection matmuls for batch b+1 are emitted interleaved
with attention groups of batch b to keep the PE queue dense (HAM warmth).
"""

import sys

for _p in ("/opt/trn_rl_repo", "/root/.axon_site/_ro/trn_rl_repo"):
    if _p not in sys.path:
        sys.path.append(_p)

import ml_dtypes
import numpy as np

import concourse.bass as bass
import concourse.tile as tile
from concourse import bacc, mybir
from concourse.bass_utils import run_bass_kernel_spmd
from concourse.masks import make_identity

dt = mybir.dt
BF16 = ml_dtypes.bfloat16

B, N, D, H, HD = 4, 2048, 1024, 16, 64
BN = B * N                      # 8192 flattened tokens
NCORES = 8
HL = H // NCORES                # 2 local heads per core
F = HL * HD                     # 128 local feats
SCALE = HD ** -0.5              # 0.125

KT = D // 128                   # 8 contraction tiles for the projections
TPB = N // 512                  # 4 token chunks per batch
KPB = N // 128                  # 16 k-tiles per batch
TT = BN // 128                  # 64 token tiles of 128
TOK = BN // NCORES              # 1024 tokens per core post-reshard
CH = N // NCORES                # 256 tokens per core per batch chunk

_compiled = None


def _build():
    nc = bacc.Bacc("TRN2", target_bir_lowering=False, debug=False,
                   num_devices=NCORES)

    f32, bf = dt.float32, dt.bfloat16

    xT = nc.declare_dram_parameter("xT", [D, BN], bf, isOutput=False)
    wqkv_t = nc.declare_dram_parameter("wqkv_t", [D, 3 * F], bf, isOutput=False)
    bqk = nc.declare_dram_parameter("bqk", [F, 2], f32, isOutput=False)
    bv = nc.declare_dram_parameter("bv", [F, 1], f32, isOutput=False)
    wout_t = nc.declare_dram_parameter("wout_t", [D, D], bf, isOutput=False)
    bout_rep = nc.declare_dram_parameter("bout_rep", [128, D], f32, isOutput=False)
    masks = nc.declare_dram_parameter("masks", [4, 128, 512], bf, isOutput=False)
    ones_col = nc.declare_dram_parameter("ones_col", [128, HL], bf, isOutput=False)
    out = nc.declare_dram_parameter("out", [TOK, D], f32, isOutput=True)

    with tile.TileContext(nc) as tc:
        with (
            tc.tile_pool(name="const", bufs=1) as const,
            tc.tile_pool(name="attn", bufs=1) as attn_pool,
            tc.tile_pool(name="dram", bufs=1, space="DRAM") as dram,
        ):
            # --- constants ---
            wqkv_sb = const.tile([128, KT, 3 * F], bf)
            for kt in range(KT):
                nc.sync.dma_start(out=wqkv_sb[:, kt, :],
                                  in_=wqkv_t[128 * kt:128 * (kt + 1), :])
            bqk_sb = const.tile([F, 2], f32)
            nc.sync.dma_start(out=bqk_sb, in_=bqk[:])
            bv_sb = const.tile([F, 1], f32)
            nc.sync.dma_start(out=bv_sb, in_=bv[:])
            masks_sb = const.tile([128, 4, 512], bf)
            for j in range(4):
                nc.sync.dma_start(out=masks_sb[:, j, :], in_=masks[j])
            ident = const.tile([128, 128], bf)
            make_identity(nc, ident)

            attnT_sb = attn_pool.tile([128, BN], bf)   # normalized O^T

            a2a_in = [dram.tile([NCORES, F, CH], bf, name=f"a2a_in{b}")
                      for b in range(B)]
            a2a_out = [dram.tile([NCORES, F, CH], bf, name=f"a2a_out{b}")
                       for b in range(B)]

            with (
                tc.tile_pool(name="qkvT", bufs=1) as qkvT,
                tc.tile_pool(name="xt", bufs=2) as xt_pool,
                tc.tile_pool(name="vt", bufs=2) as vt_pool,
                tc.tile_pool(name="pt", bufs=3) as pt_pool,
                tc.tile_pool(name="nrm", bufs=2) as nrm,
                tc.tile_pool(name="ps_qkv", bufs=2, space="PSUM") as ps_qkv,
                tc.tile_pool(name="ps_tr", bufs=1, space="PSUM") as ps_tr,
                tc.tile_pool(name="ps_s", bufs=3, space="PSUM") as ps_s,
                tc.tile_pool(name="ps_o", bufs=2, space="PSUM") as ps_o,
            ):
                qT_sb = qkvT.tile([F, BN], bf)
                kT_sb = qkvT.tile([F, BN], bf)
                v1_sb = qkvT.tile([128, TT, HL * (HD + 1)], bf)

                def proj_chunk(tch):
                    """QKV projection for one 512-token chunk."""
                    sl = slice(512 * tch, 512 * (tch + 1))
                    xt = xt_pool.tile([128, KT, 512], bf, tag="xt")
                    for kt in range(KT):
                        nc.sync.dma_start(
                            out=xt[:, kt, :],
                            in_=xT[128 * kt:128 * (kt + 1), sl])
                    for which, dst in ((0, qT_sb), (1, kT_sb)):
                        ps = ps_qkv.tile([128, 512], f32, tag="qkv")
                        for kt in range(KT):
                            nc.tensor.matmul(
                                ps,
                                wqkv_sb[:, kt, F * which:F * (which + 1)],
                                xt[:, kt, :],
                                start=(kt == 0), stop=(kt == KT - 1))
                        nc.vector.tensor_scalar_add(
                            dst[:, sl], ps, bqk_sb[:, which:which + 1])
                    ps = ps_qkv.tile([128, 512], f32, tag="qkv")
                    for kt in range(KT):
                        nc.tensor.matmul(
                            ps, wqkv_sb[:, kt, 2 * F:3 * F], xt[:, kt, :],
                            start=(kt == 0), stop=(kt == KT - 1))
                    vt = vt_pool.tile([128, 512], bf, tag="vt")
                    nc.vector.tensor_scalar_add(vt, ps, bv_sb)
                    for j in range(4):
                        tt = 4 * tch + j
                        ptr = ps_tr.tile([128, 128], bf, tag="tr")
                        nc.tensor.transpose(ptr, vt[:, 128 * j:128 * (j + 1)], ident)
                        nc.vector.tensor_copy(
                            out=v1_sb[:, tt, :].rearrange(
                                "p (h e) -> p h e", h=HL)[:, :, 0:HD],
                            in_=ptr.rearrange("p (h d) -> p h d", h=HL))
                        nc.sync.dma_start(
                            out=v1_sb[:, tt, :].rearrange(
                                "p (h e) -> p h e", h=HL)[:, :, HD:HD + 1],
                            in_=ones_col[:].unsqueeze(2))

                def attn_group(b, h, qc):
                    """Scores+softmax+PV for one (head, 512-query chunk)."""
                    hsl = slice(HD * h, HD * (h + 1))
                    qsl = slice(N * b + 512 * qc, N * b + 512 * (qc + 1))
                    po = ps_o.tile([HD + 1, 512], f32, tag="o")
                    nkt = 4 * qc + 4
                    for kt in range(nkt):
                        ks = ps_s.tile([128, 512], f32, tag="s")
                        nc.tensor.matmul(
                            ks,
                            kT_sb[hsl, N * b + 128 * kt:N * b + 128 * (kt + 1)],
                            qT_sb[hsl, qsl],
                            start=True, stop=True)
                        pt = pt_pool.tile([128, 512], bf, tag="pt")
                        nc.scalar.activation(
                            out=pt, in_=ks,
                            func=mybir.ActivationFunctionType.Exp,
                            scale=SCALE)
                        if kt >= 4 * qc:
                            nc.vector.tensor_mul(
                                pt, pt, masks_sb[:, kt - 4 * qc, :])
                        nc.tensor.matmul(
                            po,
                            v1_sb[:, KPB * b + kt,
                                  (HD + 1) * h:(HD + 1) * (h + 1)],
                            pt,
                            start=(kt == 0), stop=(kt == nkt - 1))
                    rsum = nrm.tile([1, 512], f32, tag="rsum")
                    nc.vector.tensor_copy(rsum, po[HD:HD + 1, :])
                    recip = nrm.tile([1, 512], f32, tag="recip")
                    nc.vector.reciprocal_approx_fast(recip, rsum)
                    bc = nrm.tile([HD, 512], f32, tag="bc")
                    nc.gpsimd.partition_broadcast(bc, recip)
                    nc.vector.tensor_mul(
                        attnT_sb[HD * h:HD * (h + 1), qsl],
                        po[0:HD, :], bc)

                def a2a_chunk(b):
                    """Ship batch b's attnT through the AllToAll."""
                    for j in range(NCORES):
                        c0 = N * b + CH * j
                        nc.sync.dma_start(out=a2a_in[b][j],
                                          in_=attnT_sb[:, c0:c0 + CH])
                    nc.gpsimd.collective_compute(
                        "AllToAll",
                        mybir.AluOpType.bypass,
                        replica_groups=[list(range(NCORES))],
                        ins=[a2a_in[b].opt()],
                        outs=[a2a_out[b].opt()],
                    )

                # ---- emission: proj(0), then attn(b) interleaved with
                # proj(b+1); a2a(b) fires right after attn(b) ----
                for tch in range(TPB):
                    proj_chunk(tch)
                for b in range(B):
                    groups = [(h, qc) for qc in range(TPB) for h in range(HL)]
                    for gi, (h, qc) in enumerate(groups):
                        attn_group(b, h, qc)
                        if b + 1 < B and gi % 2 == 0:
                            proj_chunk(TPB * (b + 1) + gi // 2)
                    a2a_chunk(b)

            # ---- output projection for my 1024 tokens (4 x 256 chunks) ----
            with (
                tc.tile_pool(name="oproj", bufs=1) as oproj,
                tc.tile_pool(name="osb", bufs=2) as osb,
                tc.tile_pool(name="ps_out", bufs=2, space="PSUM") as ps_out,
            ):
                wout_sb = oproj.tile([128, KT, D], bf)
                for kt in range(KT):
                    nc.sync.dma_start(out=wout_sb[:, kt, :],
                                      in_=wout_t[128 * kt:128 * (kt + 1), :])
                bout_sb = oproj.tile([128, D], f32)
                nc.sync.dma_start(out=bout_sb, in_=bout_rep[:])
                ot_sb = oproj.tile([128, KT, TOK], bf)
                for b in range(B):
                    for kt in range(KT):
                        nc.sync.dma_start(
                            out=ot_sb[:, kt, CH * b:CH * (b + 1)],
                            in_=a2a_out[b][kt])

                for mt in range(TOK // 128):
                    o_sb = osb.tile([128, D], f32, tag="osb")
                    for nb in range(2):
                        ps = ps_out.tile([128, 512], f32, tag="out")
                        for kt in range(KT):
                            nc.tensor.matmul(
                                ps,
                                ot_sb[:, kt, 128 * mt:128 * (mt + 1)],
                                wout_sb[:, kt, 512 * nb:512 * (nb + 1)],
                                start=(kt == 0), stop=(kt == KT - 1))
                        nc.vector.tensor_add(
                            o_sb[:, 512 * nb:512 * (nb + 1)], ps,
                            bout_sb[:, 512 * nb:512 * (nb + 1)])
                    nc.sync.dma_start(out=out[128 * mt:128 * (mt + 1), :], in_=o_sb)

    nc.compile()
    return nc


def _prep_inputs(x, w_qkv, b_qkv, w_out, b_out):
    x = np.asarray(x, dtype=np.float32)
    w_qkv = np.asarray(w_qkv, dtype=np.float32)
    b_qkv = np.asarray(b_qkv, dtype=np.float32)
    w_out = np.asarray(w_out, dtype=np.float32)
    b_out = np.asarray(b_out, dtype=np.float32)

    xT = np.ascontiguousarray(x.reshape(BN, D).T).astype(BF16)
    wout_t = np.ascontiguousarray(w_out.T).astype(BF16)
    bout_rep = np.ascontiguousarray(np.broadcast_to(b_out[None, :], (128, D)))
    ones_col = np.ones((128, HL), dtype=BF16)

    mk = np.zeros((4, 128, 512), dtype=np.float32)
    for j in range(4):
        kk = 128 * j + np.arange(128)[:, None]
        qq = np.arange(512)[None, :]
        mk[j] = (kk <= qq).astype(np.float32)
    mk = mk.astype(BF16)

    in_maps = []
    for i in range(NCORES):
        fs = slice(F * i, F * (i + 1))
        wq, wk, wv = w_qkv[0:D][fs], w_qkv[D:2 * D][fs], w_qkv[2 * D:3 * D][fs]
        wqkv_t = np.ascontiguousarray(
            np.concatenate([wq, wk, wv], axis=0).T).astype(BF16)
        bqk_np = np.ascontiguousarray(
            np.stack([b_qkv[0:D][fs], b_qkv[D:2 * D][fs]], axis=1))
        bv_np = np.ascontiguousarray(b_qkv[2 * D:3 * D][fs][:, None])
        in_maps.append({
            "xT": xT, "wqkv_t": wqkv_t, "bqk": bqk_np, "bv": bv_np,
            "wout_t": wout_t, "bout_rep": bout_rep, "masks": mk,
            "ones_col": ones_col,
        })
    return in_maps


def kernel(x, w_qkv, b_qkv, w_out, b_out, _results_hook=None):
    global _compiled
    if _compiled is None:
        _compiled = _build()
    in_maps = _prep_inputs(x, w_qkv, b_qkv, w_out, b_out)
    res = run_bass_kernel_spmd(_compiled, in_maps, core_ids=list(range(NCORES)))
    if _results_hook is not None:
        _results_hook(res)
    full = np.empty((B, N, D), dtype=np.float32)
    for i in range(NCORES):
        o = res.results[i]["out"]            # [1024, D]: 4 chunks of 256
        for b in range(B):
            full[b, CH * i:CH * (i + 1), :] = o[CH * b:CH * (b + 1)]
    return full


# revision 6
# speedup vs baseline: 1.5936x; 1.0385x over previous
"""Causal self-attention (B=4, N=2048, D=1024, H=16) on 8 TRN2 NeuronCores.

Sharding: head-parallel — core i computes heads {2i, 2i+1} for all batches
(QKV projection + attention), then 8-rank AllToAll collectives (one per
batch, overlapped with the next batch's attention) reshard from head-split
to token-split, and each core runs the output projection for its 1024
tokens. The AllToAll gives each core the full concat-head activation for
its tokens, so no partial-sum collective is needed.

Matmuls run in bf16 with fp32 PSUM accumulation (~3e-3 max rel error
end-to-end; bf16 streams 1 cycle/row vs ~1.8 for fp32r). Attention uses
the score-transposed (ST) layout [k, q] with 1024-wide query groups (bf16
moving operand allows N=1024) so no P transposes are needed; softmax
denominators come from a ones-column appended to V (PV matmul M=65), and
scores are ~N(0,1) so max-subtraction is unnecessary. Softmax exp on the
scalar engine is the attention pacer, so projection and output-projection
matmul bursts are emitted interleaved between attention groups to keep the
PE queue dense (HAM clock-gate warmth).
"""

import sys

for _p in ("/opt/trn_rl_repo", "/root/.axon_site/_ro/trn_rl_repo"):
    if _p not in sys.path:
        sys.path.append(_p)

import ml_dtypes
import numpy as np

import concourse.bass as bass
import concourse.tile as tile
from concourse import bacc, mybir
from concourse.bass_utils import run_bass_kernel_spmd
from concourse.masks import make_identity

dt = mybir.dt
BF16 = ml_dtypes.bfloat16

B, N, D, H, HD = 4, 2048, 1024, 16, 64
BN = B * N                      # 8192 flattened tokens
NCORES = 8
HL = H // NCORES                # 2 local heads per core
F = HL * HD                     # 128 local feats
SCALE = HD ** -0.5              # 0.125

KT = D // 128                   # 8 contraction tiles for the projections
TPB = N // 512                  # 4 512-token chunks per batch (projection)
QG = N // 1024                  # 2 1024-query groups per batch (attention)
KPB = N // 128                  # 16 k-tiles per batch
TT = BN // 128                  # 64 token tiles of 128
TOK = BN // NCORES              # 1024 tokens per core post-reshard
CH = N // NCORES                # 256 tokens per core per batch chunk

_compiled = None


def _build():
    nc = bacc.Bacc("TRN2", target_bir_lowering=False, debug=False,
                   num_devices=NCORES)

    f32, bf = dt.float32, dt.bfloat16

    xT = nc.declare_dram_parameter("xT", [D, BN], bf, isOutput=False)
    wqkv_t = nc.declare_dram_parameter("wqkv_t", [D, 3 * F], bf, isOutput=False)
    bqk = nc.declare_dram_parameter("bqk", [F, 2], f32, isOutput=False)
    bv = nc.declare_dram_parameter("bv", [F, 1], f32, isOutput=False)
    wout_t = nc.declare_dram_parameter("wout_t", [D, D], bf, isOutput=False)
    bout_rep = nc.declare_dram_parameter("bout_rep", [128, D], f32, isOutput=False)
    masks = nc.declare_dram_parameter("masks", [8, 128, 1024], bf, isOutput=False)
    ones_col = nc.declare_dram_parameter("ones_col", [128, HL], bf, isOutput=False)
    out = nc.declare_dram_parameter("out", [TOK, D], f32, isOutput=True)

    with tile.TileContext(nc) as tc:
        with (
            tc.tile_pool(name="const", bufs=1) as const,
            tc.tile_pool(name="attn", bufs=1) as attn_pool,
            tc.tile_pool(name="dram", bufs=1, space="DRAM") as dram,
            tc.tile_pool(name="qkvT", bufs=1) as qkvT,
            tc.tile_pool(name="xt", bufs=2) as xt_pool,
            tc.tile_pool(name="vt", bufs=2) as vt_pool,
            tc.tile_pool(name="pt", bufs=3) as pt_pool,
            tc.tile_pool(name="nrm", bufs=2) as nrm,
            tc.tile_pool(name="osb", bufs=2) as osb,
            tc.tile_pool(name="ps_acc", bufs=2, space="PSUM") as ps_acc,
            tc.tile_pool(name="ps_s", bufs=2, space="PSUM") as ps_s,
            tc.tile_pool(name="ps_o", bufs=1, space="PSUM") as ps_o,
        ):
            # --- constants ---
            wqkv_sb = const.tile([128, KT, 3 * F], bf)
            for kt in range(KT):
                nc.sync.dma_start(out=wqkv_sb[:, kt, :],
                                  in_=wqkv_t[128 * kt:128 * (kt + 1), :])
            bqk_sb = const.tile([F, 2], f32)
            nc.sync.dma_start(out=bqk_sb, in_=bqk[:])
            bv_sb = const.tile([F, 1], f32)
            nc.sync.dma_start(out=bv_sb, in_=bv[:])
            masks_sb = const.tile([128, 8, 1024], bf)
            for j in range(8):
                nc.sync.dma_start(out=masks_sb[:, j, :], in_=masks[j])
            ident = const.tile([128, 128], bf)
            make_identity(nc, ident)
            wout_sb = const.tile([128, KT, D], bf)
            for kt in range(KT):
                nc.sync.dma_start(out=wout_sb[:, kt, :],
                                  in_=wout_t[128 * kt:128 * (kt + 1), :])
            bout_sb = const.tile([128, D], f32)
            nc.sync.dma_start(out=bout_sb, in_=bout_rep[:])

            attnT_sb = attn_pool.tile([128, BN], bf)   # normalized O^T
            ot_sb = attn_pool.tile([128, KT, TOK], bf)  # post-A2A activations

            a2a_in = [dram.tile([NCORES, F, CH], bf, name=f"a2a_in{b}")
                      for b in range(B)]
            a2a_out = [dram.tile([NCORES, F, CH], bf, name=f"a2a_out{b}")
                       for b in range(B)]

            qT_sb = qkvT.tile([F, BN], bf)
            kT_sb = qkvT.tile([F, BN], bf)
            v1_sb = qkvT.tile([128, TT, HL * (HD + 1)], bf)

            def proj_chunk(tch):
                """QKV projection for one 512-token chunk."""
                sl = slice(512 * tch, 512 * (tch + 1))
                xt = xt_pool.tile([128, KT, 512], bf, tag="xt")
                for kt in range(KT):
                    nc.sync.dma_start(out=xt[:, kt, :],
                                      in_=xT[128 * kt:128 * (kt + 1), sl])
                for which, dst in ((0, qT_sb), (1, kT_sb)):
                    ps = ps_acc.tile([128, 512], f32, tag="acc")
                    for kt in range(KT):
                        nc.tensor.matmul(
                            ps,
                            wqkv_sb[:, kt, F * which:F * (which + 1)],
                            xt[:, kt, :],
                            start=(kt == 0), stop=(kt == KT - 1))
                    nc.vector.tensor_scalar_add(
                        dst[:, sl], ps, bqk_sb[:, which:which + 1])
                ps = ps_acc.tile([128, 512], f32, tag="acc")
                for kt in range(KT):
                    nc.tensor.matmul(
                        ps, wqkv_sb[:, kt, 2 * F:3 * F], xt[:, kt, :],
                        start=(kt == 0), stop=(kt == KT - 1))
                vt = vt_pool.tile([128, 512], bf, tag="vt")
                nc.vector.tensor_scalar_add(vt, ps, bv_sb)
                for j in range(4):
                    tt = 4 * tch + j
                    ptr = ps_acc.tile([128, 128], bf, tag="acc")
                    nc.tensor.transpose(ptr, vt[:, 128 * j:128 * (j + 1)], ident)
                    nc.vector.tensor_copy(
                        out=v1_sb[:, tt, :].rearrange(
                            "p (h e) -> p h e", h=HL)[:, :, 0:HD],
                        in_=ptr.rearrange("p (h d) -> p h d", h=HL))
                    nc.sync.dma_start(
                        out=v1_sb[:, tt, :].rearrange(
                            "p (h e) -> p h e", h=HL)[:, :, HD:HD + 1],
                        in_=ones_col[:].unsqueeze(2))

            def attn_group(b, h, qg):
                """Scores+softmax+PV for one (head, 1024-query group)."""
                hsl = slice(HD * h, HD * (h + 1))
                qsl = slice(N * b + 1024 * qg, N * b + 1024 * (qg + 1))
                po = ps_o.tile([HD + 1, 1024], f32, tag="o")
                nkt = 8 * qg + 8
                q0 = N * b + 1024 * qg
                for kt in range(nkt):
                    ks = ps_s.tile([128, 1024], f32, tag="s")
                    for half in range(2):
                        nc.tensor.matmul(
                            ks[:, 512 * half:512 * (half + 1)],
                            kT_sb[hsl, N * b + 128 * kt:N * b + 128 * (kt + 1)],
                            qT_sb[hsl, q0 + 512 * half:q0 + 512 * (half + 1)],
                            start=True, stop=True)
                    pt = pt_pool.tile([128, 1024], bf, tag="pt")
                    nc.scalar.activation(
                        out=pt, in_=ks,
                        func=mybir.ActivationFunctionType.Exp,
                        scale=SCALE)
                    if kt >= 8 * qg:
                        nc.vector.tensor_mul(
                            pt, pt, masks_sb[:, kt - 8 * qg, :])
                    for half in range(2):
                        nc.tensor.matmul(
                            po[:, 512 * half:512 * (half + 1)],
                            v1_sb[:, KPB * b + kt,
                                  (HD + 1) * h:(HD + 1) * (h + 1)],
                            pt[:, 512 * half:512 * (half + 1)],
                            start=(kt == 0), stop=(kt == nkt - 1))
                rsum = nrm.tile([1, 1024], f32, tag="rsum")
                nc.vector.tensor_copy(rsum, po[HD:HD + 1, :])
                recip = nrm.tile([1, 1024], f32, tag="recip")
                nc.vector.reciprocal_approx_fast(recip, rsum)
                bc = nrm.tile([HD, 1024], f32, tag="bc")
                nc.gpsimd.partition_broadcast(bc, recip)
                nc.vector.tensor_mul(
                    attnT_sb[HD * h:HD * (h + 1), qsl], po[0:HD, :], bc)

            def a2a_chunk(b):
                """Ship batch b's attnT through the AllToAll."""
                for j in range(NCORES):
                    c0 = N * b + CH * j
                    nc.sync.dma_start(out=a2a_in[b][j],
                                      in_=attnT_sb[:, c0:c0 + CH])
                nc.gpsimd.collective_compute(
                    "AllToAll",
                    mybir.AluOpType.bypass,
                    replica_groups=[list(range(NCORES))],
                    ins=[a2a_in[b].opt()],
                    outs=[a2a_out[b].opt()],
                )
                for kt in range(KT):
                    nc.sync.dma_start(
                        out=ot_sb[:, kt, CH * b:CH * (b + 1)],
                        in_=a2a_out[b][kt])

            def outproj_mt(mt):
                """Output projection for one 128-token tile."""
                o_sb = osb.tile([128, D], f32, tag="osb")
                for nb in range(2):
                    ps = ps_acc.tile([128, 512], f32, tag="acc")
                    for kt in range(KT):
                        nc.tensor.matmul(
                            ps,
                            ot_sb[:, kt, 128 * mt:128 * (mt + 1)],
                            wout_sb[:, kt, 512 * nb:512 * (nb + 1)],
                            start=(kt == 0), stop=(kt == KT - 1))
                    nc.vector.tensor_add(
                        o_sb[:, 512 * nb:512 * (nb + 1)], ps,
                        bout_sb[:, 512 * nb:512 * (nb + 1)])
                nc.sync.dma_start(out=out[128 * mt:128 * (mt + 1), :], in_=o_sb)

            # ---- emission schedule ----
            # proj(0) dense; per batch b: 4 attention groups with one
            # proj chunk of b+1 after each, and outproj tiles of batch b-1
            # (post-A2A) slotted mid-batch; a2a(b) fires right after the
            # batch's last attention group.
            for tch in range(TPB):
                proj_chunk(tch)
            for b in range(B):
                groups = [(h, qg) for qg in range(QG) for h in range(HL)]
                for gi, (h, qg) in enumerate(groups):
                    attn_group(b, h, qg)
                    if b + 1 < B:
                        proj_chunk(TPB * (b + 1) + gi)
                    if b >= 1 and gi % 2 == 1:
                        outproj_mt(2 * (b - 1) + gi // 2)
                a2a_chunk(b)
            for mt in (6, 7):
                outproj_mt(mt)

    nc.compile()
    return nc


def _prep_inputs(x, w_qkv, b_qkv, w_out, b_out):
    x = np.asarray(x, dtype=np.float32)
    w_qkv = np.asarray(w_qkv, dtype=np.float32)
    b_qkv = np.asarray(b_qkv, dtype=np.float32)
    w_out = np.asarray(w_out, dtype=np.float32)
    b_out = np.asarray(b_out, dtype=np.float32)

    xT = np.ascontiguousarray(x.reshape(BN, D).T).astype(BF16)
    wout_t = np.ascontiguousarray(w_out.T).astype(BF16)
    bout_rep = np.ascontiguousarray(np.broadcast_to(b_out[None, :], (128, D)))
    ones_col = np.ones((128, HL), dtype=BF16)

    mk = np.zeros((8, 128, 1024), dtype=np.float32)
    for j in range(8):
        kk = 128 * j + np.arange(128)[:, None]
        qq = np.arange(1024)[None, :]
        mk[j] = (kk <= qq).astype(np.float32)
    mk = mk.astype(BF16)

    in_maps = []
    for i in range(NCORES):
        fs = slice(F * i, F * (i + 1))
        wq, wk, wv = w_qkv[0:D][fs], w_qkv[D:2 * D][fs], w_qkv[2 * D:3 * D][fs]
        wqkv_t = np.ascontiguousarray(
            np.concatenate([wq, wk, wv], axis=0).T).astype(BF16)
        bqk_np = np.ascontiguousarray(
            np.stack([b_qkv[0:D][fs], b_qkv[D:2 * D][fs]], axis=1))
        bv_np = np.ascontiguousarray(b_qkv[2 * D:3 * D][fs][:, None])
        in_maps.append({
            "xT": xT, "wqkv_t": wqkv_t, "bqk": bqk_np, "bv": bv_np,
            "wout_t": wout_t, "bout_rep": bout_rep, "masks": mk,
            "ones_col": ones_col,
        })
    return in_maps


def kernel(x, w_qkv, b_qkv, w_out, b_out, _results_hook=None):
    global _compiled
    if _compiled is None:
        _compiled = _build()
    in_maps = _prep_inputs(x, w_qkv, b_qkv, w_out, b_out)
    res = run_bass_kernel_spmd(_compiled, in_maps, core_ids=list(range(NCORES)))
    if _results_hook is not None:
        _results_hook(res)
    full = np.empty((B, N, D), dtype=np.float32)
    for i in range(NCORES):
        o = res.results[i]["out"]            # [1024, D]: 4 chunks of 256
        for b in range(B):
            full[b, CH * i:CH * (i + 1), :] = o[CH * b:CH * (b + 1)]
    return full
